# revision 42
# baseline (speedup 1.0000x reference)
"""Trainium2 Bass kernel for nn_ConvLogicLayer.

Computes y[n,c,oy,ox,p] = k0 + ka*A + kb*B + kab*A*B where A/B are
shifted-window gathers of input channels (per the packed `selection`),
and k* are per-(c,p) coefficients derived from softmax(weights) @ OP_COEFFS.

Strategy (fp16, custom fused DVE op; model HW exec ~84.4us vs 95.4us
baseline, engines balanced at ~85-90% busy, DMA 73%):
  - Shard C_out (512) across 8 cores -> 64 output channels per core.
  - Per-core specialized programs: gather indices and coefficients baked
    into the instruction stream (static access patterns + immediates).
  - A custom DVE op MULADD_STT_ANT (out = (in0 + s0)*in1 + s1, registered
    into concourse.dve_ops at import; per-NEFF uop table ships via the
    bass2jax HLO frontend-attribute path) collapses each pair to TWO ops:
        u = kab*B + ka        (tensor_scalar, any engine)
        y = (A + c)*u + r     (custom DVE, 1x: 327ns/[128,256])
    with c = kb/kab, r = k0 - ka*kb/kab (exact algebra; the custom op
    evaluates in fp32 internally, one fp16 rounding at the output).
  - ~11% of pairs run as split chains to unload DVE: u, s = A + c on
    ACT/GPS slots, product t = s*u as a batched GPSIMD tensor_tensor,
    y = t + r on ACT/GPS -- zero DVE time for those pairs. The water-level
    LP picks the split count; split pairs go in the trailing yc slots
    (host unpermutes via pair_inv) and spread mid-stream + tail.
  - |kab| <= kd pairs drop the product (err <= kd):
        uf = ka*A + k0; y = (kb*B) + uf  via AFFINE_THEN_ADD custom.
    min(|ka|,|kb|) > cthr*|kab| pairs (f1) use feeder + 2 customs.
  - Engine slots for tensor_scalar work (feeders, split-chain s/y) are
    assigned by a water-level LP + error diffusion across DVE/ACT/GPS,
    with sigma supply-slack so feeders run slightly ahead of DVE demand.
  - Emission is software-pipelined: produce(feeders+split products) runs
    `prefetch` channels ahead of finish(customs+split y+DMA), so DVE's
    custom stream never head-of-line blocks on a late feeder. The first
    `warm_self` channels self-feed on DVE (ACT/GPS ramp slowly); the last
    `tail_split` channels each put up to 2 pairs on split chains so all
    three engines drain together. Knobs are auto-tuned per core over an
    80-point grid via TimelineSim (~0.1s per build+sim).
  - One output DMA per channel: each extra dma_start costs 625ns of HWDGE
    descriptor-gen, which measurably beats any finer-grained overlap.
  - Input channels are HOST-PERMUTED per core in greedy discovery order,
    loaded as a cascade of small contiguous DMAs; output channels are
    emitted in that order so compute starts ~3.8us in (input-latency
    floor) and overlaps the load.
  - Output HBM layout is [q=(n,oyblk), cl, slot, oy', ox] fp16; the host
    transposes/unpermutes/upcasts to [N, C, H, W, 4] f32.
"""

import os
import re
import sys
import threading

import numpy as np

for _p in ("/opt/trn_rl_repo",):
    if _p not in sys.path and os.path.isdir(_p):
        sys.path.insert(0, _p)

import concourse.bass as bass
import concourse.bacc as bacc
import concourse.mybir as mybir
from concourse.tile import TileContext
from concourse import bass_utils
from concourse import dve_ops as _dve_ops_mod
from concourse.dve_ops import DveOp
from concourse.dve_spec import Spec, Src0, Src1, C0, C1

# ---------------------------------------------------------------------------
# Custom DVE op registration: out = (in0 + s0) * in1 + s1. Idempotent.
# ---------------------------------------------------------------------------


def _register_muladd_stt():
    name = "MULADD_STT_ANT"
    for op in _dve_ops_mod.OPS:
        if op.name == name:
            return op
    op = DveOp.__new__(DveOp)
    object.__setattr__(op, "name", name)
    object.__setattr__(
        op,
        "spec",
        Spec(
            body=(Src0 + C0) * Src1 + C1,
            reference=lambda in0, in1, s0, s1, imm2: (
                (in0.astype(np.float32) + s0) * in1 + s1
            ),
        ),
    )
    object.__setattr__(op, "subdim", False)
    object.__setattr__(op, "uops_sha", {})
    object.__setattr__(op, "perf_en", {})
    _dve_ops_mod.OPS.append(op)
    _dve_ops_mod.CUSTOM_DVE_SPECS[name] = op.spec
    _dve_ops_mod._SUB_OPCODE_FOR_NAME[name] = (
        _dve_ops_mod._CUSTOM_DVE_ROW_BASE + len(_dve_ops_mod.OPS) - 1
    )
    assert _dve_ops_mod._SUB_OPCODE_FOR_NAME[name] < 0x20
    # pin the sha by compiling once and catching the declared-vs-got error
    for ver in ("v3", "v4"):
        try:
            op.compile(ver)
        except ValueError as e:
            m = re.search(r"(\w{16}) ≠ pinned", str(e))
            if not m:
                raise
            op.uops_sha[ver] = m.group(1)
            op.compile(ver)
    return op


MULADD_STT = _register_muladd_stt()
AFFINE_THEN_ADD = _dve_ops_mod.AFFINE_THEN_ADD

# Problem constants (hardcoded per spec)
N, C_IN, H, W = 32, 64, 32, 32
C_OUT, KPAIRS = 512, 4
N_CORES = 8
CPC = C_OUT // N_CORES  # channels per core

P = 128          # partitions = (n=32) x (oyblk=4)
OYB = 4          # oy blocks per image
OYS = 8          # oy rows per block
HALO = 10        # rows stored per block (8 + 2 halo)
W34 = 34         # padded width
CHSZ = HALO * W34           # 340 elems per (q, channel)
XFREE = C_IN * CHSZ         # 21760 elems per partition
TILE = OYS * W              # 256 elems per pair tile (per partition)
CH_OUT = KPAIRS * TILE      # 1024 out elems per channel per partition

OP_COEFFS = np.array([
    [0.0, 0.0, 0.0, 0.0], [0.0, 0.0, 0.0, 1.0], [0.0, 1.0, 0.0, -1.0],
    [0.0, 1.0, 0.0, 0.0], [0.0, 0.0, 1.0, -1.0], [0.0, 0.0, 1.0, 0.0],
    [0.0, 1.0, 1.0, -2.0], [0.0, 1.0, 1.0, -1.0], [1.0, -1.0, -1.0, 1.0],
    [1.0, -1.0, -1.0, 2.0], [1.0, 0.0, -1.0, 0.0], [1.0, 0.0, -1.0, 1.0],
    [1.0, -1.0, 0.0, 0.0], [1.0, -1.0, 0.0, 1.0], [1.0, 0.0, 0.0, -1.0],
    [1.0, 0.0, 0.0, 0.0],
], dtype=np.float64)

MULT = mybir.AluOpType.mult
ADD = mybir.AluOpType.add
COPY = mybir.ActivationFunctionType.Copy
F16 = mybir.dt.float16
F32 = mybir.dt.float32

# Cost-model ns per op on a [128, 256] tile (hw_specs + v2 visitors)
C_DVE_TS16 = 127.1   # tensor_scalar fp16 4x_2p
C_DVE_CUST = 327.1   # custom DVE (1x), also scalar_tensor_tensor
C_ACT_TS = 398.3     # activation copy w/ scale+bias (dtype-independent)
C_GPS_TS = 450.6     # gpsimd tensor_scalar (0.6 eff + 95ns launch)
C_GPS_TT4 = 2127.2   # gpsimd tensor_tensor mult [128,1024] (0.42 eff)

CFG = {
    "cthr": 20.0,         # |c| bound for the factorized custom form
    "kab_drop": 3e-3,     # |kab| below which the product term is dropped
    "cascade": (4, 4, 4, 4, 6, 6, 6, 6, 6, 6, 6),  # load DMA sizes
    "tp_bufs": 48,
    "yc_bufs": 12,
    "ub_bufs": 3,
    "disc_tb": 0,    # discovery-order tie-break variant
    "prefetch": 6,   # channels of feeder lookahead
    "sigma": 1.03,   # supply-slack: helper TS planned slower than modeled
    "tail_split": 4, # last K channels get (up to 2) split pairs each
    "warm_self": 2,  # first E channels self-feed their customs on DVE
    "zmul": 1.0,
}

last_results = [None] * N_CORES  # BassKernelResults per core (for profiling)
last_model_ns = [None] * N_CORES  # per-core TimelineSim estimate


def _pair_forms(coef, c0):
    """Classify each (cl, p4) of channels [c0, c0+CPC) into a compute form.

    Returns dict (cl, p4) -> tuple:
      ("f2a", kab, ka, c, r)   u = kab*B + ka;  y = (A + c)*u + r
      ("f2b", kab, kb, c, r)   u = kab*A + kb;  y = (B + c)*u + r
      ("f3", k0, ka, kb)       uf = ka*A + k0;  y = (kb*B) + uf
      ("f1", k0, ka, kb, kab)  u = kab*B + ka; t = A*u + k0; y = kb*B + t
    """
    cthr = CFG["cthr"]
    kd = CFG["kab_drop"]
    forms = {}
    for cl in range(CPC):
        for p4 in range(KPAIRS):
            k0, ka, kb, kab = (float(coef[c0 + cl, p4, j]) for j in range(4))
            if abs(kab) <= kd:
                forms[(cl, p4)] = ("f3", k0, ka, kb)
            elif min(abs(ka), abs(kb)) <= cthr * abs(kab):
                r = k0 - ka * kb / kab
                if abs(kb) <= abs(ka):
                    forms[(cl, p4)] = ("f2a", kab, ka, kb / kab, r)
                else:
                    forms[(cl, p4)] = ("f2b", kab, kb, ka / kab, r)
            else:
                forms[(cl, p4)] = ("f1", k0, ka, kb, kab)
    return forms


def _core_layout(core, ch):
    """Per-core channel permutation + load tiers + channel order.

    Greedy discovery order: repeatedly pick the output channel needing the
    fewest not-yet-loaded input channels; its new inputs are appended to
    the permutation. The host writes xh with channels in this order, so
    the load is a sequence of small contiguous DMAs and output channels
    become fully processable roughly linearly in time.
    """
    c0 = core * CPC
    need = {
        cl: {int(ch[c0 + cl, k]) for k in range(2 * KPAIRS)}
        for cl in range(CPC)
    }
    use_cnt = [0] * C_IN
    for cl in range(CPC):
        for k in range(2 * KPAIRS):
            use_cnt[int(ch[c0 + cl, k])] += 1

    loaded = set()
    perm = []
    cl_order = []
    cums = []
    remaining = set(range(CPC))
    sgn = -1 if CFG.get("disc_tb", 0) == 0 else 1
    while remaining:
        best = min(
            remaining,
            key=lambda cl: (
                len(need[cl] - loaded),
                sgn * sum(use_cnt[i] for i in need[cl] - loaded),
            ),
        )
        for i in sorted(need[best] - loaded, key=lambda i: -use_cnt[i]):
            perm.append(i)
            loaded.add(i)
        cl_order.append(best)
        remaining.remove(best)
        cums.append(len(perm))
    newpos = {orig: i for i, orig in enumerate(perm)}

    if CFG["cascade"] == "adapt":
        # tier boundaries aligned to the first few channels' cumulative
        # input needs (each becomes computable the moment its tier lands),
        # then steady 4-channel tiers
        bounds = []
        for b in cums[:6]:
            if b < C_IN and (not bounds or b > bounds[-1]):
                bounds.append(b)
        while bounds[-1] < C_IN:
            bounds.append(min(C_IN, bounds[-1] + 4))
    else:
        sizes = list(CFG["cascade"])
        sizes.append(C_IN - sum(sizes))
        sizes = [s for s in sizes if s > 0]
        bounds = np.cumsum(sizes).tolist()
    return perm, newpos, cl_order, bounds


def build_core_program(core, ch, ry, rx, coef):
    """One specialized Bass program for `core` (channels core*CPC..+CPC)."""
    nc = bacc.Bacc("TRN2", target_bir_lowering=False)
    xh_d = nc.dram_tensor("xh", [P, XFREE], F16, kind="ExternalInput")
    out_d = nc.dram_tensor("out", [P, CPC * CH_OUT], F16, kind="ExternalOutput")

    c0 = core * CPC
    forms = _pair_forms(coef, c0)
    perm, newpos, cl_order, bounds = _core_layout(core, ch)

    def pair_ready(cl, p4):
        return max(newpos[int(ch[c0 + cl, 2 * p4])],
                   newpos[int(ch[c0 + cl, 2 * p4 + 1])])

    p4_order = {
        cl: sorted(range(KPAIRS), key=lambda p4: pair_ready(cl, p4))
        for cl in range(CPC)
    }
    pair_inv = [[0] * KPAIRS for _ in range(CPC)]  # [cl][p4] -> yc slot

    # ---- quota planning: water-level LP + schedule-shaping knobs --------
    nf2 = sum(1 for f in forms.values() if f[0].startswith("f2"))
    nf1 = sum(1 for f in forms.values() if f[0] == "f1")
    nf3 = sum(1 for f in forms.values() if f[0] == "f3")
    sigma = CFG.get("sigma", 1.06)      # supply-slack: helpers planned slower
    tail_k = CFG.get("tail_split", 6)   # last K channels emit split chains
    warm_e = CFG.get("warm_self", 5)    # first E channels self-feed on DVE

    # tail channels: alternate f2 pairs into split chains (cap 2/channel) so
    # DVE, ACT and GPS drain together without a GPS-TT pile-up at the end.
    # Optionally keep the very last channel(s) unsplit so their DVE-serial
    # completions stagger the final output DMAs instead of piling up.
    split_set = set()
    t_excl = CFG.get("tail_excl_last", 0)
    tail_cls = cl_order[CPC - tail_k - t_excl: CPC - t_excl] if tail_k else []
    for cl in tail_cls:
        f2s = [p4 for p4 in range(KPAIRS)
               if forms[(cl, p4)][0].startswith("f2")]
        for p4 in f2s[::2][:2]:
            split_set.add((cl, p4))
    z_tail = len(split_set)

    def plan(z):
        """Bisect minimal T for z extra split pairs; returns (T, fractions)."""
        zt = z + z_tail
        d_fix = C_DVE_CUST * (nf2 - zt + nf3) + 2 * C_DVE_CUST * nf1
        g_fix = (C_GPS_TT4 / 4.0) * zt
        n_ts = nf2 + nf3 + nf1 - zt + 3 * zt
        lo_t, hi_t = max(d_fix, g_fix), 6.0e5
        for _ in range(60):
            mid = 0.5 * (lo_t + hi_t)
            cap = (
                max(0.0, mid - d_fix) / C_DVE_TS16
                + mid / (C_ACT_TS * sigma)
                + max(0.0, mid - g_fix) / (C_GPS_TS * sigma)
            )
            if cap >= n_ts:
                hi_t = mid
            else:
                lo_t = mid
        T = hi_t
        n_d = max(0.0, T - d_fix) / C_DVE_TS16
        n_a = T / (C_ACT_TS * sigma)
        n_g = max(0.0, T - g_fix) / (C_GPS_TS * sigma)
        return T, (n_d, n_a, n_g)

    z_best, T_best, fr = 0, None, None
    for z in range(0, nf2 - z_tail + 1, 4):
        T, f = plan(z)
        if T_best is None or T < T_best:
            z_best, T_best, fr = z, T, f
    z_best = min(int(round(z_best * CFG.get("zmul", 1.0))), nf2 - z_tail)
    T_best, fr = plan(z_best)
    n_d, n_a, n_g = fr
    ntot = max(n_d + n_a + n_g, 1e-9)
    frac = {"dve": n_d / ntot, "act": n_a / ntot, "gps": n_g / ntot}
    acc = {k: 0.0 for k in frac}
    real = {k: 0 for k in frac}

    def slot_pick(allowed=("dve", "act", "gps")):
        for k in frac:
            acc[k] += frac[k]
        key = max(allowed, key=lambda k: acc[k] - real[k])
        real[key] += 1
        return key

    # mid-stream split pairs: spread evenly, skipping warmup and tail
    emit_seq = []
    for cl in cl_order[warm_e: CPC - tail_k if tail_k else CPC]:
        for p4 in p4_order[cl]:
            if forms[(cl, p4)][0].startswith("f2"):
                emit_seq.append((cl, p4))
    if z_best > 0 and emit_seq:
        step = max(1, len(emit_seq) // z_best)
        added = 0
        for i in range(0, len(emit_seq), step):
            if added >= z_best:
                break
            split_set.add(emit_seq[i])
            added += 1
    warm_set = set(cl_order[:warm_e])

    with TileContext(nc) as tc:
        with (
            tc.tile_pool(name="xp", bufs=1) as xpool,
            tc.tile_pool(name="tp", bufs=CFG["tp_bufs"]) as tpool,
            tc.tile_pool(name="yp", bufs=CFG["yc_bufs"]) as ypool,
        ):
            xh = xpool.tile([P, XFREE], F16)
            lo = 0
            for b in bounds:
                hi = b * CHSZ
                nc.sync.dma_start(xh[:, lo:hi], xh_d[:, lo:hi])
                lo = hi

            base = xh[:]
            pitch = base.ap[0][0]
            tens = base.tensor
            base_off = base.offset

            def win(c, k):
                o = (base_off + newpos[int(ch[c, k])] * CHSZ
                     + int(ry[c, k]) * W34 + int(rx[c, k]))
                return bass.AP(tens, o, [[pitch, P], [W34, OYS], [1, W]])

            def do_ts(eng, out3, in3, scale, bias):
                # out = scale*in + bias (fp16)
                if eng == "dve":
                    if scale == 1.0:
                        nc.vector.tensor_scalar(out3, in3, bias, None, ADD)
                    else:
                        nc.vector.tensor_scalar(out3, in3, scale, bias, MULT, ADD)
                elif eng == "act":
                    nc.scalar.activation(out3, in3, COPY, bias=bias, scale=scale)
                else:
                    if scale == 1.0:
                        nc.gpsimd.tensor_scalar(out3, in3, bias, None, ADD)
                    else:
                        nc.gpsimd.tensor_scalar(out3, in3, scale, bias, MULT, ADD)

            def new_t(tag):
                t = tpool.tile([P, TILE], F16, tag=tag)
                return t[:], t[:].rearrange("p (a b) -> p a b", b=W)

            BT = KPAIRS * TILE
            state = {}  # cl -> (ufs dict p4->tile, tb, ch_splits)
            split_ctr = [0]  # split-channel counter for TT engine rotation

            def produce(cl):
                """Feeders + split-pair batched GPS product for channel cl."""
                c = c0 + cl
                warm = cl in warm_set
                ch_splits = [p4 for p4 in p4_order[cl]
                             if (cl, p4) in split_set]
                n_sp = len(ch_splits)
                tb = None
                if n_sp:
                    ub = tpool.tile([P, BT], F16, tag="ub", bufs=CFG.get("ub_bufs", 3))
                    sb = tpool.tile([P, BT], F16, tag="sb", bufs=CFG.get("ub_bufs", 3))
                    tb = tpool.tile([P, BT], F16, tag="tb", bufs=CFG.get("ub_bufs", 3))
                    for i, p4 in enumerate(ch_splits):
                        form = forms[(cl, p4)]
                        _, kab, klin, cc, r = form
                        ka_, kb_ = 2 * p4, 2 * p4 + 1
                        A_ap, B_ap = win(c, ka_), win(c, kb_)
                        uin, tin = (B_ap, A_ap) if form[0] == "f2a" else (A_ap, B_ap)
                        u3 = ub[:, i * TILE:(i + 1) * TILE].rearrange(
                            "p (a b) -> p a b", b=W)
                        s3 = sb[:, i * TILE:(i + 1) * TILE].rearrange(
                            "p (a b) -> p a b", b=W)
                        do_ts(slot_pick(("act", "gps")), u3, uin, kab, klin)
                        do_ts(slot_pick(("act", "gps")), s3, tin, 1.0, cc)
                    L = n_sp * TILE
                    tmod = CFG.get("tt_dve_mod", 0)
                    split_ctr[0] += 1
                    if tmod and split_ctr[0] % tmod == 0:
                        nc.vector.tensor_tensor(
                            tb[:, :L], sb[:, :L], ub[:, :L], MULT)
                    else:
                        nc.gpsimd.tensor_tensor(
                            tb[:, :L], sb[:, :L], ub[:, :L], MULT)
                ufs = {}
                for p4 in p4_order[cl]:
                    if (cl, p4) in split_set:
                        continue
                    form = forms[(cl, p4)]
                    ka_, kb_ = 2 * p4, 2 * p4 + 1
                    A_ap, B_ap = win(c, ka_), win(c, kb_)
                    uf, u3 = new_t("u")
                    eng = "dve" if warm else slot_pick()
                    if form[0] in ("f2a", "f2b"):
                        _, kab, klin, cc, r = form
                        uin = B_ap if form[0] == "f2a" else A_ap
                        do_ts(eng, u3, uin, kab, klin)
                    elif form[0] == "f3":
                        _, k0, ka, kb = form
                        do_ts(eng, u3, A_ap, ka, k0)
                    else:  # f1
                        _, k0, ka, kb, kab = form
                        do_ts(eng, u3, B_ap, kab, ka)
                    ufs[p4] = uf
                state[cl] = (ufs, tb, ch_splits)

            def finish(cl, gidx):
                """Customs + split-pair y adds + output DMA for channel cl.

                Custom pairs land in the leading yc slots and DMA out as
                soon as they're done; split pairs (longer cross-engine
                chains) fill the trailing slots with their own DMA, so the
                custom stream never waits on a split chain. The host undoes
                the per-channel pair permutation (pair_inv).
                """
                c = c0 + cl
                ufs, tb, ch_splits = state.pop(cl)
                ch_customs = [p4 for p4 in p4_order[cl] if p4 not in ch_splits]
                for j, p4 in enumerate(ch_customs + ch_splits):
                    pair_inv[cl][p4] = j
                yc = ypool.tile([P, CH_OUT], F16, tag="yc")
                for j, p4 in enumerate(ch_customs):
                    form = forms[(cl, p4)]
                    ka_, kb_ = 2 * p4, 2 * p4 + 1
                    yoff = j * TILE
                    y3 = yc[:, yoff: yoff + TILE].rearrange(
                        "p (a b) -> p a b", b=W)
                    A_ap, B_ap = win(c, ka_), win(c, kb_)
                    if form[0] in ("f2a", "f2b"):
                        _, kab, klin, cc, r = form
                        tin = A_ap if form[0] == "f2a" else B_ap
                        nc.vector._custom_dve(
                            MULADD_STT, out=y3, in0=tin, in1=ufs[p4],
                            s0=float(cc), s1=float(r),
                        )
                    elif form[0] == "f3":
                        _, k0, ka, kb = form
                        nc.vector._custom_dve(
                            AFFINE_THEN_ADD, out=y3, in0=B_ap, in1=ufs[p4],
                            s0=float(kb), s1=0.0,
                        )
                    else:  # f1
                        _, k0, ka, kb, kab = form
                        tf, t3 = new_t("t")
                        nc.vector._custom_dve(
                            MULADD_STT, out=t3, in0=A_ap, in1=ufs[p4],
                            s0=0.0, s1=float(k0),
                        )
                        nc.vector._custom_dve(
                            AFFINE_THEN_ADD, out=y3, in0=B_ap, in1=tf,
                            s0=float(kb), s1=0.0,
                        )
                ncust = len(ch_customs)
                for i, p4 in enumerate(ch_splits):
                    r = forms[(cl, p4)][4]
                    yoff = (ncust + i) * TILE
                    do_ts(slot_pick(("act", "gps")), yc[:, yoff: yoff + TILE],
                          tb[:, i * TILE:(i + 1) * TILE], 1.0, r)
                # One DMA per channel: extra per-channel DMAs cost 625ns of
                # HWDGE descriptor-gen each and measurably slow the timeline.
                # Exception: the last tail_pair_dma channels stream per-slot
                # DMAs so the final transfer is short (HWDGE is idle then).
                if gidx >= CPC - CFG.get("tail_pair_dma", 0):
                    for j in range(KPAIRS):
                        oap = bass.AP(
                            out_d, gidx * CH_OUT + j * TILE,
                            [[CPC * CH_OUT, P], [1, TILE]],
                        )
                        nc.sync.dma_start(oap, yc[:, j * TILE:(j + 1) * TILE])
                else:
                    oap = bass.AP(
                        out_d, gidx * CH_OUT,
                        [[CPC * CH_OUT, P], [1, CH_OUT]],
                    )
                    nc.sync.dma_start(oap, yc[:])

            D = CFG.get("prefetch", 1)
            for i in range(min(D, CPC)):
                produce(cl_order[i])
            for gidx, cl in enumerate(cl_order):
                if gidx + D < CPC:
                    produce(cl_order[gidx + D])
                finish(cl, gidx)
    nc.finalize()  # Bacc: splits >1-wait syncs into event semaphores
    nc._pair_inv = np.asarray(pair_inv)  # host-side unpermute of yc slots
    return nc


def _prep_inputs(x, weights, selection):
    x = np.ascontiguousarray(np.asarray(x, dtype=np.float32))
    weights = np.asarray(weights, dtype=np.float32)
    selection = np.asarray(selection, dtype=np.int32)

    # coefficients: softmax over 16 logic ops folded into {1,a,b,ab} basis
    w64 = weights.astype(np.float64)
    e = np.exp(w64 - w64.max(axis=-1, keepdims=True))
    prob = e / e.sum(axis=-1, keepdims=True)
    coef = (prob @ OP_COEFFS).astype(np.float32)  # [C_OUT, 4, 4]

    ch = ((selection >> 16) & 0xFFFF).astype(np.int64)
    ry = ((selection >> 8) & 0xFF).astype(np.int64)
    rx = (selection & 0xFF).astype(np.int64)

    # halo layout: xh[q=(n,oyblk), ch, r, w] = xpad[n, ch, oyblk*8+r, w]
    xpad = np.zeros((N, C_IN, H + 2, W + 2), dtype=np.float32)
    xpad[:, :, 1: H + 1, 1: W + 1] = x
    xh = np.empty((N, OYB, C_IN, HALO, W34), dtype=np.float16)
    for b in range(OYB):
        xh[:, b] = xpad[:, :, b * OYS: b * OYS + HALO, :]
    xh = xh.reshape(P, C_IN, CHSZ)
    return xh, ch, ry, rx, coef


def kernel(x, weights, selection):
    assert x.shape == (N, C_IN, H, W), x.shape
    assert weights.shape == (C_OUT, 4, 16), weights.shape
    assert selection.shape == (C_OUT, 8), selection.shape

    xh3, ch, ry, rx, coef = _prep_inputs(x, weights, selection)

    # Per-core auto-tune over (start offsets, discovery tiebreak) via
    # TimelineSim. The tiebreak changes the channel permutation, so the
    # host input arrays are built AFTER selection from the winner's layout.
    try:
        from concourse.timeline_sim import TimelineSim
    except Exception:  # noqa: BLE001
        TimelineSim = None
    import itertools as _it

    cas_a = (4, 4, 4, 4, 6, 6, 6, 6, 6, 6, 6)
    cas_b = (3, 3, 4, 4, 4, 6, 6, 6, 6, 6, 6, 6)
    cas_e = (4,) * 15
    cands = CFG.get("tune_candidates") or tuple(
        # (cascade, disc_tb, sigma, tail_split, warm_self, tail_excl) --
        # neighborhood of the winners from wider offline sweeps
        (cas, tb, sg, tk, we, tex)
        for cas, tb, sg, tk, we, tex in _it.product(
            (cas_e, cas_b, cas_a), (0, 1), (1.03, 1.06), (4, 6), (1, 2, 3),
            (0, 1),
        )
    )
    progs = []
    base = dict(CFG)
    xh_arrs = []
    cl_orders = []
    for k in range(N_CORES):
        best = None
        for cas, tb, sg, tk, we, tex in cands if TimelineSim is not None else (
            (CFG["cascade"], CFG["disc_tb"], CFG["sigma"], CFG["tail_split"],
             CFG["warm_self"], CFG.get("tail_excl_last", 0)),
        ):
            CFG["cascade"] = cas
            CFG["disc_tb"] = tb
            CFG["sigma"] = sg
            CFG["tail_split"] = tk
            CFG["warm_self"] = we
            CFG["tail_excl_last"] = tex
            nc = build_core_program(k, ch, ry, rx, coef)
            ns = None
            if TimelineSim is not None:
                try:
                    ns = TimelineSim(nc, trace=False).simulate()
                except Exception:  # noqa: BLE001
                    ns = None
            if best is None or (ns is not None and best[0] is not None and ns < best[0]):
                best = (ns, nc, cas, tb)
            if ns is None:
                break
        progs.append(best[1])
        last_model_ns[k] = best[0]
        # the host input layout must match the WINNER's discovery order
        CFG["cascade"], CFG["disc_tb"] = best[2], best[3]
        perm, _, cl_order, _ = _core_layout(k, ch)
        xh_arrs.append(np.ascontiguousarray(xh3[:, perm].reshape(P, XFREE)))
        cl_orders.append(np.asarray(cl_order))
    CFG.update(base)

    import jax

    devices = jax.devices()
    assert len(devices) >= N_CORES, devices

    y = np.empty((N, C_OUT, H, W, KPAIRS), dtype=np.float32)
    errs = [None] * N_CORES
    # NTFF tracing needs axon hooks that aren't present in this container —
    # make sure run_bass_kernel_spmd never tries (BASS_TRACE in env would).
    os.environ["BASS_NEVER_TRACE"] = "1"

    def run_one(k):
        try:
            with jax.default_device(devices[k]):
                res = bass_utils.run_bass_kernel_spmd(
                    progs[k], [{"xh": xh_arrs[k]}], core_ids=[k]
                )
            last_results[k] = res
            buf = res.results[0]["out"]  # [P, CPC*1024] fp16
            # [q=(n,blk), j, slot, r, w] -> [n, cl_order[j], blk*8+r, w, p4]
            b6 = buf.reshape(N, OYB, CPC, KPAIRS, OYS, W)
            b7 = b6.transpose(0, 2, 1, 4, 5, 3).reshape(N, CPC, H, W, KPAIRS)
            inv_emit = progs[k]._pair_inv[cl_orders[k]]  # [CPC, 4] p4->slot
            y[:, k * CPC + cl_orders[k]] = np.take_along_axis(
                b7, inv_emit[None, :, None, None, :], axis=-1
            ).astype(np.float32)
        except Exception as e:  # noqa: BLE001
            errs[k] = e

    threads = [threading.Thread(target=run_one, args=(k,)) for k in range(N_CORES)]
    for t in threads:
        t.start()
    for t in threads:
        t.join()
    for k, e in enumerate(errs):
        if e is not None:
            raise RuntimeError(f"core {k} failed") from e
    return y
